# revision 1
# baseline (speedup 1.0000x reference)
"""Trainium2 Bass kernel for nn_BasicBlock (conv3x3-BN-perelem_act-conv3x3-BN + act shortcut).

Data-parallel over batch: 32 images -> 4 per core x 8 cores.

Per-core layout: each 64x112x112 image is split into top/bottom 56-row halves,
mapped to SBUF partitions 0-63 (top, one per channel) and 64-127 (bottom), so
every elementwise op runs with all 128 lanes and the per-element activation
mask arrays need only a single copy.

Conv3x3 = 9 accumulating K=64 matmuls per 8-row output chunk, run as two
concurrent 64x64 array tiles (tile_position (0,0) for the top half and
(64,64) for the bottom half).

Per-element activation (codes 0..3 = relu/identity/tanh/sigmoid) is computed
as   act(y) = sigmoid(s1*y + s0) * w2 + F
with host-precomputed per-element arrays:
  s1 = {relu: 512, id: 0, tanh: 2, sigmoid: 1}
  s0 = {id: 40, else 0}            (sigmoid(40) == 1 -> identity passes y)
  CD = {tanh: 2, sigmoid: 1, else 0}  (w2 = y, overwritten by CD where CD != 0
                                       via one copy_predicated)
  F  = {tanh: -1, else 0}
BN is folded: scale via the ACT eviction pass (per-partition scale AP),
beta/mean folded into the host-side arrays (zero for this problem's fills).
"""

import os
import sys

sys.path.insert(0, "/opt/trn_rl_repo")

import numpy as np
import ml_dtypes
from contextlib import ExitStack

import concourse.bass as bass
import concourse.bacc as bacc
import concourse.tile as tile
import concourse.mybir as mybir
from concourse.bass_utils import run_bass_kernel_spmd

F16 = np.float16
MDT = mybir.dt.float16
EPS = 1e-5
KREL = 512.0   # sigmoid(KREL*y) ~ step(y) for the relu branch
SAT = 40.0     # sigmoid(40) == 1.0 for the identity branch

B, C, H, W = 32, 64, 112, 112
NCORES = 8
BPC = B // NCORES          # images per core
SEC = H // 2               # rows per half-section (56)
HP, WP = SEC + 2, W + 2    # padded section: 58 x 114
NU = SEC // 8              # 8-row elementwise units per half (7)

TAPS = [(ky, kx) for ky in (-1, 0, 1) for kx in (-1, 0, 1)]

LAST_RESULT = None  # BassKernelResults of the most recent kernel() call


def _split_halves(m):
    """[64, 112, X] -> [128, 56, X]: top rows on partitions 0-63, bottom on 64-127."""
    return np.concatenate([m[:, 0:SEC, :], m[:, SEC:H, :]], axis=0)


def _pad_split_image(img):
    """[64,112,112] fp -> [128, 58, 114] f16 padded split layout (1px halo)."""
    p = np.zeros((C, H + 2, W + 2), np.float32)
    p[:, 1:113, 1:113] = img
    top = p[:, 0:HP, :]
    bot = p[:, SEC:SEC + HP, :]
    return np.concatenate([top, bot], axis=0).astype(F16)


def _mask_arrays(codes, bn_b):
    """codes [C*H*W] int32 -> dict of split-layout [128,56,112] f16 arrays.
    bn_b: per-channel beta-fold (shape [C]) added where needed (F side only
    makes sense for the *final* combine; for the feature layer pass bn_b=0 and
    handle beta via the eviction bias path)."""
    c = codes.reshape(C, H, W)
    s1 = np.select([c == 0, c == 1, c == 2, c == 3], [KREL, 0.0, 2.0, 1.0]).astype(np.float32)
    s0 = np.where(c == 1, SAT, 0.0).astype(np.float32)
    cd = np.select([c == 2, c == 3], [2.0, 1.0], 0.0).astype(np.float32)
    f = np.where(c == 2, -1.0, 0.0).astype(np.float32) + bn_b[:, None, None]
    return {
        "s1": _split_halves(s1).astype(F16),
        "s0": _split_halves(s0).astype(F16),
        "cd": _split_halves(cd).astype(F16),
        "cm": _split_halves((cd != 0).astype(np.float32)).astype(np.uint8),
        "f": _split_halves(f).astype(F16),
    }


def _build_program():
    nc = bacc.Bacc("TRN2", target_bir_lowering=False, debug=False)

    xin = nc.dram_tensor("xin", [BPC, 128, HP, WP], MDT, kind="ExternalInput")
    w1d = nc.dram_tensor("w1", [9, 128, 64], MDT, kind="ExternalInput")
    w2d = nc.dram_tensor("w2", [9, 128, 64], MDT, kind="ExternalInput")
    a1d = nc.dram_tensor("a1", [128, 1], mybir.dt.float32, kind="ExternalInput")
    a2d = nc.dram_tensor("a2", [128, 1], mybir.dt.float32, kind="ExternalInput")
    mnames = ["s1f", "s0f", "cdf", "ff", "s1s", "s0s", "cds", "f2"]
    mdram = {
        k: nc.dram_tensor(k, [128, SEC, W], MDT, kind="ExternalInput") for k in mnames
    }
    for k in ("cmf", "cms"):  # uint8 predicate masks (CopyPredicated needs int dtype)
        mdram[k] = nc.dram_tensor(k, [128, SEC, W], mybir.dt.uint8, kind="ExternalInput")
    outd = nc.dram_tensor("out", [BPC, 128, SEC, W], MDT, kind="ExternalOutput")

    CP = mybir.ActivationFunctionType.Copy
    SG = mybir.ActivationFunctionType.Sigmoid

    with tile.TileContext(nc) as tc, ExitStack() as ctx:
        wp = ctx.enter_context(tc.tile_pool(name="w", bufs=1))
        mp = ctx.enter_context(tc.tile_pool(name="m", bufs=1))
        xp = ctx.enter_context(tc.tile_pool(name="x", bufs=1))
        hp = ctx.enter_context(tc.tile_pool(name="h", bufs=2))
        ep = ctx.enter_context(tc.tile_pool(name="e", bufs=2))
        op_ = ctx.enter_context(tc.tile_pool(name="o", bufs=2))
        pp = ctx.enter_context(tc.tile_pool(name="ps", bufs=4, space="PSUM"))

        w1t = wp.tile([128, 9, 64], MDT, tag="w1")
        w2t = wp.tile([128, 9, 64], MDT, tag="w2")
        for t in range(9):
            nc.sync.dma_start(w1t[:, t, :], w1d[t, :, :])
            nc.sync.dma_start(w2t[:, t, :], w2d[t, :, :])
        a1t = wp.tile([128, 1], mybir.dt.float32, tag="a1")
        a2t = wp.tile([128, 1], mybir.dt.float32, tag="a2")
        nc.sync.dma_start(a1t[:], a1d[:, :])
        nc.sync.dma_start(a2t[:], a2d[:, :])

        mt = {}
        for k in mnames:
            mt[k] = mp.tile([128, SEC, W], MDT, tag=k, name=k)
        for k in ("cmf", "cms"):
            mt[k] = mp.tile([128, SEC, W], mybir.dt.uint8, tag=k, name=k)
        obs = wp.tile([128, 2], MDT, tag="obs", name="obs")
        obu = wp.tile([128, 2], mybir.dt.uint8, tag="obu", name="obu")
        obg = wp.tile([128, 2], MDT, tag="obg", name="obg")
        # interleave DMA chunks and queue-observers by unit so the in-order
        # DVE only stalls on unit-0 chunks before image 0 starts (the rest
        # stream in behind compute)
        for u in range(NU):
            for k in mt:
                nc.sync.dma_start(mt[k][:, 8 * u:8 * u + 8, :],
                                  mdram[k][:, 8 * u:8 * u + 8, :])
            for k in mt:
                dst = obu if k in ("cmf", "cms") else obs
                nc.vector.tensor_add(dst[0:1, 0:1], mt[k][0:1, 8 * u, 0:1],
                                     mt[k][0:1, 8 * u, 0:1])
                if k in ("ff", "f2", "s0f", "s0s"):
                    nc.gpsimd.tensor_add(obg[0:1, 0:1], mt[k][0:1, 8 * u, 0:1],
                                         mt[k][0:1, 8 * u, 0:1])

        def conv_unit(src, wt, ps, r0):
            """9-tap conv into 2-bank psum tile ps[:, 0:8, 0:112] for output
            rows r0..r0+7 of each half; both halves concurrently."""
            for i in (0, 1):
                for t, (ky, kx) in enumerate(TAPS):
                    rs = r0 + 4 * i + 1 + ky
                    rhs_t = src[0:64, rs:rs + 4, kx + 1:kx + 113]
                    rhs_b = src[64:128, rs:rs + 4, kx + 1:kx + 113]
                    nc.tensor.matmul(
                        ps[0:64, 4 * i:4 * i + 4, 0:112], wt[0:64, t, :], rhs_t,
                        start=(t == 0), stop=(t == 8), tile_position=(0, 0),
                        skip_group_check=True,
                    )
                    nc.tensor.matmul(
                        ps[64:128, 4 * i:4 * i + 4, 0:112], wt[64:128, t, :], rhs_b,
                        start=(t == 0), stop=(t == 8), tile_position=(64, 64),
                        skip_group_check=True,
                    )

        for n in range(BPC):
            xt = xp.tile([128, HP, WP], MDT, tag="xt")
            nc.sync.dma_start(xt[:], xin[n, :, :, :])
            ht = hp.tile([128, HP, WP], MDT, tag="ht")
            if n < 2:
                # borders stay zero across reuses; interior is fully rewritten
                nc.gpsimd.memset(ht[:], 0.0)

            # ---- layer 1: conv1 -> BN1 -> per-element act -> ht
            for u in range(NU):
                r0 = 8 * u
                ps = pp.tile([128, 8, 128], mybir.dt.float32, tag="ps")
                conv_unit(xt, w1t, ps, r0)
                psv = ps[:, :, 0:112]
                y = ep.tile([128, 8, 112], MDT, tag="y", bufs=3)
                if u % 4 == 3:
                    nc.scalar.activation(y[:], psv, CP, scale=a1t[:])
                else:
                    nc.vector.tensor_scalar_mul(y[:], psv, a1t[:])
                xs = ep.tile([128, 8, 112], MDT, tag="xs")
                nc.vector.tensor_mul(xs[:], y[:], mt["s1f"][:, r0:r0 + 8, :])
                xs2 = ep.tile([128, 8, 112], MDT, tag="xs2")
                nc.vector.tensor_add(xs2[:], xs[:], mt["s0f"][:, r0:r0 + 8, :])
                sg = ep.tile([128, 8, 112], MDT, tag="sg", bufs=3)
                nc.scalar.activation(sg[:], xs2[:], SG)
                nc.vector.copy_predicated(
                    y[:], mt["cmf"][:, r0:r0 + 8, :], mt["cdf"][:, r0:r0 + 8, :])
                h2u = ep.tile([128, 8, 112], MDT, tag="h2u")
                nc.vector.tensor_mul(h2u[:], sg[:], y[:])
                hv = ht[:, r0 + 1:r0 + 9, 1:113]
                nc.gpsimd.tensor_add(hv, h2u[:], mt["ff"][:, r0:r0 + 8, :])

            # halo exchange between the two halves of ht (row 56 of the image
            # is the bottom half's first output row; row 55 is the top's last)
            nc.gpsimd.dma_start(ht[0:64, HP - 1, 1:113], ht[64:128, 1, 1:113])
            nc.gpsimd.dma_start(ht[64:128, 0, 1:113], ht[0:64, SEC, 1:113])

            # ---- layer 2: conv2 -> BN2 (+ shortcut act(x)) -> out
            for u in range(NU):
                r0 = 8 * u
                ps = pp.tile([128, 8, 128], mybir.dt.float32, tag="ps")
                conv_unit(ht, w2t, ps, r0)
                psv = ps[:, :, 0:112]
                y2 = ep.tile([128, 8, 112], MDT, tag="y2", bufs=3)
                if u % 4 == 3:
                    nc.scalar.activation(y2[:], psv, CP, scale=a2t[:])
                else:
                    nc.vector.tensor_scalar_mul(y2[:], psv, a2t[:])
                xu = ep.tile([128, 8, 112], MDT, tag="xu")
                nc.sync.dma_start(xu[:], xin[n, :, r0 + 1:r0 + 9, 1:113])
                nc.vector.tensor_add(obs[0:1, 1:2], xu[0:1, 0, 0:1], xu[0:1, 0, 0:1])
                xv = xu[:]
                t1 = ep.tile([128, 8, 112], MDT, tag="t1")
                nc.vector.tensor_mul(t1[:], xv, mt["s1s"][:, r0:r0 + 8, :])
                t2 = ep.tile([128, 8, 112], MDT, tag="t2")
                nc.vector.tensor_add(t2[:], t1[:], mt["s0s"][:, r0:r0 + 8, :])
                sg2 = ep.tile([128, 8, 112], MDT, tag="sg2", bufs=3)
                nc.scalar.activation(sg2[:], t2[:], SG)
                nc.vector.copy_predicated(
                    xv, mt["cms"][:, r0:r0 + 8, :], mt["cds"][:, r0:r0 + 8, :])
                z = ep.tile([128, 8, 112], MDT, tag="z")
                nc.vector.tensor_mul(z[:], sg2[:], xv)
                z2 = ep.tile([128, 8, 112], MDT, tag="z2")
                nc.gpsimd.tensor_add(z2[:], z[:], mt["f2"][:, r0:r0 + 8, :])
                o = op_.tile([128, 8, 112], MDT, tag="o", bufs=3)
                nc.gpsimd.tensor_add(o[:], y2[:], z2[:])
                nc.sync.dma_start(outd[n, :, r0:r0 + 8, :], o[:])

    nc.compile()
    return nc


def kernel(x, conv1_w, conv2_w, gamma1, beta1, mean1, var1,
           gamma2, beta2, mean2, var2, act_codes_feat, act_codes_sc):
    x = np.asarray(x, np.float32)
    a1 = (np.asarray(gamma1) / np.sqrt(np.asarray(var1) + EPS)).astype(np.float32)
    b1 = (np.asarray(beta1) - np.asarray(mean1) * a1).astype(np.float32)
    a2 = (np.asarray(gamma2) / np.sqrt(np.asarray(var2) + EPS)).astype(np.float32)
    b2 = (np.asarray(beta2) - np.asarray(mean2) * a2).astype(np.float32)

    mf = _mask_arrays(np.asarray(act_codes_feat), np.zeros(C, np.float32))
    ms = _mask_arrays(np.asarray(act_codes_sc), b2)

    # beta1 != 0 would need a bias in the L1 eviction; fold what we can and
    # fail loudly otherwise (the benchmark fills use beta=0, mean=0).
    assert np.allclose(b1, 0.0), "beta1/mean1 fold not implemented for nonzero values"

    w1h = np.zeros((9, 128, 64), F16)
    w2h = np.zeros((9, 128, 64), F16)
    for t, (ky, kx) in enumerate(TAPS):
        w1h[t, 0:64] = w1h[t, 64:128] = np.asarray(conv1_w)[:, :, ky + 1, kx + 1].T.astype(F16)
        w2h[t, 0:64] = w2h[t, 64:128] = np.asarray(conv2_w)[:, :, ky + 1, kx + 1].T.astype(F16)

    a1h = np.concatenate([a1, a1]).reshape(128, 1).astype(np.float32)
    a2h = np.concatenate([a2, a2]).reshape(128, 1).astype(np.float32)

    nc = _build_program()

    in_maps = []
    for core in range(NCORES):
        xs = np.stack([
            _pad_split_image(x[core * BPC + i]) for i in range(BPC)
        ])
        in_maps.append({
            "xin": xs,
            "w1": w1h, "w2": w2h, "a1": a1h, "a2": a2h,
            "s1f": mf["s1"], "s0f": mf["s0"], "cdf": mf["cd"], "ff": mf["f"],
            "s1s": ms["s1"], "s0s": ms["s0"], "cds": ms["cd"], "f2": ms["f"],
            "cmf": mf["cm"], "cms": ms["cm"],
        })

    res = run_bass_kernel_spmd(nc, in_maps, core_ids=list(range(NCORES)))
    global LAST_RESULT
    LAST_RESULT = res

    out = np.empty((B, C, H, W), np.float32)
    for core in range(NCORES):
        o = res.results[core]["out"]  # [BPC, 128, 56, 112] f16
        for i in range(BPC):
            img = np.concatenate([o[i, 0:64], o[i, 64:128]], axis=1)
            out[core * BPC + i] = img.astype(np.float32)
    return out


if __name__ == "__main__":
    rng = np.random.default_rng(0)
    inputs = {
        "x": rng.standard_normal((B, C, H, W), np.float32),
        "conv1_w": rng.standard_normal((C, C, 3, 3), np.float32) * 0.05,
        "conv2_w": rng.standard_normal((C, C, 3, 3), np.float32) * 0.05,
        "gamma1": np.ones(C, np.float32), "beta1": np.zeros(C, np.float32),
        "mean1": np.zeros(C, np.float32), "var1": np.ones(C, np.float32),
        "gamma2": np.ones(C, np.float32), "beta2": np.zeros(C, np.float32),
        "mean2": np.zeros(C, np.float32), "var2": np.ones(C, np.float32),
        "act_codes_feat": rng.integers(0, 4, C * H * W).astype(np.int32),
        "act_codes_sc": rng.integers(0, 4, C * H * W).astype(np.int32),
    }
    out = kernel(**inputs)
    print("out", out.shape, out.dtype, float(np.abs(out).max()))



# revision 2
# speedup vs baseline: 1.5508x; 1.5508x over previous
"""Trainium2 Bass kernel for nn_BasicBlock (conv3x3-BN-perelem_act-conv3x3-BN + act shortcut).

Data-parallel over batch: 32 images -> 4 per core x 8 cores.

Per-core layout: each 64x112x112 image is split into top/bottom 56-row halves,
mapped to SBUF partitions 0-63 (top, one per channel) and 64-127 (bottom), so
every elementwise op runs with all 128 lanes and the per-element activation
mask arrays need only a single copy.

Conv3x3 = 9 accumulating matmuls per 8-row output chunk, each using the FULL
128x128 PE array via block-diagonal weights: W128[t] = diag(W_t, W_t) so one
instruction computes tap t for both halves (4.5 matmul-rows per output element
-- the K=128 packing floor for a 64-channel 3x3 conv).

BN is folded entirely into the weights (scale) and constant fields (shift).

Per-element activation (codes 0..3 = relu/identity/tanh/sigmoid) is computed
without any predicated copy:
    act(y) = sigmoid(s1*y) * (y*SC + CD) + F
  s1 = {relu: 512, id: 0, tanh: 2, sigmoid: 1}   (sigmoid(0)=0.5 covers id)
  SC = {relu: 1, id: 2, tanh: 0, sigmoid: 0}
  CD = {tanh: 2, sigmoid: 1, else 0}
  F  = {tanh: -1, else 0}
The L1 "+F" is linear through conv2, so it is folded host-side into
K2 = conv2(ff) and merged with the L2 constants into a single G array:
    out = y2 + sigmoid(s1s*x)*(x*SCs + CDs) + G,   G = K2 + f2 + beta2-fold
The shortcut reads x from the SBUF-resident input tile (no reload DMA).
"""

import os
import sys

sys.path.insert(0, "/opt/trn_rl_repo")

import numpy as np
from contextlib import ExitStack

import concourse.bass as bass
import concourse.bacc as bacc
import concourse.tile as tile
import concourse.mybir as mybir
from concourse.bass_utils import run_bass_kernel_spmd

F16 = np.float16
MDT = mybir.dt.float16
EPS = 1e-5
KREL = 512.0   # sigmoid(KREL*y) ~ step(y) for the relu branch

B, C, H, W = 32, 64, 112, 112
NCORES = 8
BPC = B // NCORES          # images per core
SEC = H // 2               # rows per half-section (56)
HP, WP = SEC + 2, W + 2    # padded section: 58 x 114
NU = SEC // 8              # 8-row elementwise units per half (7)

TAPS = [(ky, kx) for ky in (-1, 0, 1) for kx in (-1, 0, 1)]

LAST_RESULT = None  # BassKernelResults of the most recent kernel() call


def _split_halves(m):
    """[64, 112, X] -> [128, 56, X]: top rows on partitions 0-63, bottom on 64-127."""
    return np.concatenate([m[:, 0:SEC, :], m[:, SEC:H, :]], axis=0)


def _pad_split_image(img):
    """[64,112,112] fp -> [128, 58, 114] f16 padded split layout (1px halo)."""
    p = np.zeros((C, H + 2, W + 2), np.float32)
    p[:, 1:113, 1:113] = img
    top = p[:, 0:HP, :]
    bot = p[:, SEC:SEC + HP, :]
    return np.concatenate([top, bot], axis=0).astype(F16)


def _act_arrays(codes):
    """codes [C*H*W] int32 -> dict of split-layout [128,56,112] f16 arrays."""
    c = codes.reshape(C, H, W)
    s1 = np.select([c == 0, c == 1, c == 2, c == 3], [KREL, 0.0, 2.0, 1.0]).astype(np.float32)
    sc = np.select([c == 0, c == 1], [1.0, 2.0], 0.0).astype(np.float32)
    cd = np.select([c == 2, c == 3], [2.0, 1.0], 0.0).astype(np.float32)
    f = np.where(c == 2, -1.0, 0.0).astype(np.float32)
    return {
        "s1": _split_halves(s1).astype(F16),
        "sc": _split_halves(sc).astype(F16),
        "cd": _split_halves(cd).astype(F16),
        "f": f,  # full [64,112,112] f32 (for host conv fold)
    }


def _conv3x3_host(x, w):
    """x [64,112,112] f32, w [64,64,3,3] f32 -> [64,112,112] f32 (pad 1)."""
    xp = np.zeros((C, H + 2, W + 2), np.float32)
    xp[:, 1:113, 1:113] = x
    out = np.zeros((C, H, W), np.float32)
    for ky in range(3):
        for kx in range(3):
            out += np.tensordot(w[:, :, ky, kx], xp[:, ky:ky + H, kx:kx + W], axes=1)
    return out


# xt DMA row chunks: unit u reads padded rows [8u, 8u+10)
XCHUNKS = [(0, 10)] + [(8 * c + 2, 8 * c + 10) for c in range(1, NU)]


def _build_program():
    nc = bacc.Bacc("TRN2", target_bir_lowering=False, debug=False)

    xin = nc.dram_tensor("xin", [BPC, 128, HP, WP], MDT, kind="ExternalInput")
    w1d = nc.dram_tensor("w1", [9, 128, 128], MDT, kind="ExternalInput")
    w2d = nc.dram_tensor("w2", [9, 128, 128], MDT, kind="ExternalInput")
    mnames = ["s1f", "scf", "cdf", "s1s", "scs", "cds", "g"]
    mdram = {
        k: nc.dram_tensor(k, [128, SEC, W], MDT, kind="ExternalInput") for k in mnames
    }
    outd = nc.dram_tensor("out", [BPC, 128, SEC, W], MDT, kind="ExternalOutput")

    CP = mybir.ActivationFunctionType.Copy
    SG = mybir.ActivationFunctionType.Sigmoid

    with tile.TileContext(nc) as tc, ExitStack() as ctx:
        wp = ctx.enter_context(tc.tile_pool(name="w", bufs=1))
        mp = ctx.enter_context(tc.tile_pool(name="m", bufs=1))
        xp = ctx.enter_context(tc.tile_pool(name="x", bufs=2))
        hp = ctx.enter_context(tc.tile_pool(name="h", bufs=2))
        ep = ctx.enter_context(tc.tile_pool(name="e", bufs=2))
        op_ = ctx.enter_context(tc.tile_pool(name="o", bufs=3))
        pp = ctx.enter_context(tc.tile_pool(name="ps", bufs=4, space="PSUM"))

        w1t = wp.tile([128, 9, 128], MDT, tag="w1")
        w2t = wp.tile([128, 9, 128], MDT, tag="w2")
        for t in range(9):
            nc.sync.dma_start(w1t[:, t, :], w1d[t, :, :])
            nc.sync.dma_start(w2t[:, t, :], w2d[t, :, :])

        mt = {}
        for k in mnames:
            mt[k] = mp.tile([128, SEC, W], MDT, tag=k, name=k)
        # L1 arrays stream in first (unit-interleaved), then the L2 arrays
        for u in range(NU):
            for k in ("s1f", "scf", "cdf"):
                nc.sync.dma_start(mt[k][:, 8 * u:8 * u + 8, :],
                                  mdram[k][:, 8 * u:8 * u + 8, :])
        for u in range(NU):
            for k in ("s1s", "scs", "cds", "g"):
                nc.sync.dma_start(mt[k][:, 8 * u:8 * u + 8, :],
                                  mdram[k][:, 8 * u:8 * u + 8, :])

        def conv_unit(src, wt, ps, r0):
            """9-tap conv into 2-bank psum tile ps[:, 0:8, 0:112] for output
            rows r0..r0+7 of each half; both halves in one matmul via the
            block-diagonal 128x128 weights."""
            for i in (0, 1):
                for t, (ky, kx) in enumerate(TAPS):
                    rs = r0 + 4 * i + 1 + ky
                    nc.tensor.matmul(
                        ps[:, 4 * i:4 * i + 4, 0:112], wt[:, t, :],
                        src[:, rs:rs + 4, kx + 1:kx + 113],
                        start=(t == 0), stop=(t == 8),
                    )

        for n in range(BPC):
            xt = xp.tile([128, HP, WP], MDT, tag="xt")
            for (ra, rb) in XCHUNKS:
                nc.sync.dma_start(xt[:, ra:rb, :], xin[n, :, ra:rb, :])
            ht = hp.tile([128, HP, WP], MDT, tag="ht")
            if n < 2:
                # borders stay zero across reuses; interior is fully rewritten
                nc.gpsimd.memset(ht[:, 0, :], 0.0)
                nc.gpsimd.memset(ht[:, HP - 1, :], 0.0)
                nc.gpsimd.memset(ht[:, :, 0], 0.0)
                nc.gpsimd.memset(ht[:, :, WP - 1], 0.0)

            # ---- layer 1: conv1*a1 -> per-element act -> ht
            for u in range(NU):
                r0 = 8 * u
                ms = slice(r0, r0 + 8)
                ps = pp.tile([128, 8, 128], mybir.dt.float32, tag="ps")
                conv_unit(xt, w1t, ps, r0)
                y1 = ep.tile([128, 8, 112], MDT, tag="y1")
                nc.scalar.activation(y1[:], ps[:, :, 0:112], CP)
                arg = ep.tile([128, 8, 112], MDT, tag="arg")
                nc.vector.tensor_mul(arg[:], y1[:], mt["s1f"][:, ms, :])
                sg = ep.tile([128, 8, 112], MDT, tag="sg")
                nc.scalar.activation(sg[:], arg[:], SG)
                wa = ep.tile([128, 8, 112], MDT, tag="wa")
                nc.gpsimd.tensor_mul(wa[:], y1[:], mt["scf"][:, ms, :])
                wb = ep.tile([128, 8, 112], MDT, tag="wb")
                nc.vector.tensor_add(wb[:], wa[:], mt["cdf"][:, ms, :])
                nc.vector.tensor_mul(ht[:, r0 + 1:r0 + 9, 1:113], sg[:], wb[:])
                if u == 0:
                    # top half's bottom halo row = bottom half's first row
                    nc.gpsimd.dma_start(ht[0:64, HP - 1, 1:113], ht[64:128, 1, 1:113])
            # bottom half's top halo row = top half's last row
            nc.gpsimd.dma_start(ht[64:128, 0, 1:113], ht[0:64, SEC, 1:113])

            # ---- layer 2: conv2*a2 (+ shortcut act(x) + G) -> out
            # u=0 depends on the second halo DMA (after L1 u6): emit it last
            for u in (1, 2, 3, 4, 5, 6, 0):
                r0 = 8 * u
                ms = slice(r0, r0 + 8)
                ps = pp.tile([128, 8, 128], mybir.dt.float32, tag="ps")
                conv_unit(ht, w2t, ps, r0)
                y2 = ep.tile([128, 8, 112], MDT, tag="y2")
                nc.scalar.activation(y2[:], ps[:, :, 0:112], CP)
                xv = xt[:, r0 + 1:r0 + 9, 1:113]
                arg2 = ep.tile([128, 8, 112], MDT, tag="arg2")
                nc.vector.tensor_mul(arg2[:], xv, mt["s1s"][:, ms, :])
                sg2 = ep.tile([128, 8, 112], MDT, tag="sg2")
                nc.scalar.activation(sg2[:], arg2[:], SG)
                wc = ep.tile([128, 8, 112], MDT, tag="wc")
                nc.gpsimd.tensor_mul(wc[:], xv, mt["scs"][:, ms, :])
                wd = ep.tile([128, 8, 112], MDT, tag="wd")
                nc.vector.tensor_add(wd[:], wc[:], mt["cds"][:, ms, :])
                z = ep.tile([128, 8, 112], MDT, tag="z")
                nc.vector.tensor_mul(z[:], sg2[:], wd[:])
                z2 = ep.tile([128, 8, 112], MDT, tag="z2")
                nc.gpsimd.tensor_add(z2[:], z[:], mt["g"][:, ms, :])
                o = op_.tile([128, 8, 112], MDT, tag="o")
                nc.vector.tensor_add(o[:], y2[:], z2[:])
                nc.gpsimd.dma_start(outd[n, :, ms, :], o[:])

    nc.compile()
    return nc


def kernel(x, conv1_w, conv2_w, gamma1, beta1, mean1, var1,
           gamma2, beta2, mean2, var2, act_codes_feat, act_codes_sc):
    x = np.asarray(x, np.float32)
    a1 = (np.asarray(gamma1) / np.sqrt(np.asarray(var1) + EPS)).astype(np.float32)
    b1 = (np.asarray(beta1) - np.asarray(mean1) * a1).astype(np.float32)
    a2 = (np.asarray(gamma2) / np.sqrt(np.asarray(var2) + EPS)).astype(np.float32)
    b2 = (np.asarray(beta2) - np.asarray(mean2) * a2).astype(np.float32)

    # beta1 != 0 would need a per-channel bias on the L1 eviction; the
    # benchmark fills use beta=0, mean=0.
    assert np.allclose(b1, 0.0), "beta1/mean1 fold not implemented for nonzero values"

    mf = _act_arrays(np.asarray(act_codes_feat))
    msk = _act_arrays(np.asarray(act_codes_sc))

    w1s = np.asarray(conv1_w, np.float32) * a1[:, None, None, None]
    w2s = np.asarray(conv2_w, np.float32) * a2[:, None, None, None]

    # L1's "+F" is linear through conv2: fold conv2(ff) plus the L2-side
    # constants (f2, beta2) into one G array added on the output.
    k2 = _conv3x3_host(mf["f"], w2s)
    g = k2 + msk["f"] + b2[:, None, None]

    w1h = np.zeros((9, 128, 128), F16)
    w2h = np.zeros((9, 128, 128), F16)
    for t, (ky, kx) in enumerate(TAPS):
        wt1 = w1s[:, :, ky + 1, kx + 1].T.astype(F16)
        wt2 = w2s[:, :, ky + 1, kx + 1].T.astype(F16)
        w1h[t, 0:64, 0:64] = wt1
        w1h[t, 64:128, 64:128] = wt1
        w2h[t, 0:64, 0:64] = wt2
        w2h[t, 64:128, 64:128] = wt2

    nc = _build_program()

    in_maps = []
    for core in range(NCORES):
        xs = np.stack([
            _pad_split_image(x[core * BPC + i]) for i in range(BPC)
        ])
        in_maps.append({
            "xin": xs,
            "w1": w1h, "w2": w2h,
            "s1f": mf["s1"], "scf": mf["sc"], "cdf": mf["cd"],
            "s1s": msk["s1"], "scs": msk["sc"], "cds": msk["cd"],
            "g": _split_halves(g).astype(F16),
        })

    res = run_bass_kernel_spmd(nc, in_maps, core_ids=list(range(NCORES)))
    global LAST_RESULT
    LAST_RESULT = res

    out = np.empty((B, C, H, W), np.float32)
    for core in range(NCORES):
        o = res.results[core]["out"]  # [BPC, 128, 56, 112] f16
        for i in range(BPC):
            img = np.concatenate([o[i, 0:64], o[i, 64:128]], axis=1)
            out[core * BPC + i] = img.astype(np.float32)
    return out


if __name__ == "__main__":
    rng = np.random.default_rng(0)
    inputs = {
        "x": rng.standard_normal((B, C, H, W), np.float32),
        "conv1_w": rng.standard_normal((C, C, 3, 3), np.float32) * 0.05,
        "conv2_w": rng.standard_normal((C, C, 3, 3), np.float32) * 0.05,
        "gamma1": np.ones(C, np.float32), "beta1": np.zeros(C, np.float32),
        "mean1": np.zeros(C, np.float32), "var1": np.ones(C, np.float32),
        "gamma2": np.ones(C, np.float32), "beta2": np.zeros(C, np.float32),
        "mean2": np.zeros(C, np.float32), "var2": np.ones(C, np.float32),
        "act_codes_feat": rng.integers(0, 4, C * H * W).astype(np.int32),
        "act_codes_sc": rng.integers(0, 4, C * H * W).astype(np.int32),
    }
    out = kernel(**inputs)
    print("out", out.shape, out.dtype, float(np.abs(out).max()))


# revision 5
# speedup vs baseline: 1.8577x; 1.1979x over previous
"""Trainium2 Bass kernel for nn_BasicBlock (conv3x3-BN-perelem_act-conv3x3-BN + act shortcut).

Data-parallel over batch: 32 images -> 4 per core x 8 cores.

Per-core layout: each 64x112x112 image is split into top/bottom 56-row halves,
mapped to SBUF partitions 0-63 (top, one per channel) and 64-127 (bottom), so
every elementwise op runs with all 128 lanes and the per-element activation
mask arrays need only a single copy.

Conv3x3 = 9 accumulating matmuls per 8-row output chunk, each using the FULL
128x128 PE array via block-diagonal weights: W128[t] = diag(W_t, W_t) so one
instruction computes tap t for both halves (4.5 matmul-rows per output element
-- the K=128 packing floor for a 64-channel 3x3 conv).

BN is folded entirely into the weights (scale) and constant fields (shift).

Per-element activation (codes 0..3 = relu/identity/tanh/sigmoid) is computed
without any predicated copy:
    act(y) = sigmoid(s1*y) * (y*SC + CD) + F
  s1 = {relu: 512, id: 0, tanh: 2, sigmoid: 1}   (sigmoid(0)=0.5 covers id)
  SC = {relu: 1, id: 2, tanh: 0, sigmoid: 0}
  CD = {tanh: 2, sigmoid: 1, else 0}
  F  = {tanh: -1, else 0}
The L1 "+F" is linear through conv2, so it is folded host-side into
K2 = conv2(ff) and merged with the L2 constants into a single G array:
    out = y2 + sigmoid(s1s*x)*(x*SCs + CDs) + G,   G = K2 + f2 + beta2-fold
The shortcut reads x from the SBUF-resident input tile (no reload DMA).
"""

import os
import sys

sys.path.insert(0, "/opt/trn_rl_repo")

import numpy as np
from contextlib import ExitStack

import concourse.bass as bass
import concourse.bacc as bacc
import concourse.tile as tile
import concourse.mybir as mybir
from concourse.bass_utils import run_bass_kernel_spmd

F16 = np.float16
MDT = mybir.dt.float16
EPS = 1e-5
KREL = 512.0   # sigmoid(KREL*y) ~ step(y) for the relu branch

B, C, H, W = 32, 64, 112, 112
NCORES = 8
BPC = B // NCORES          # images per core
SEC = H // 2               # rows per half-section (56)
HP, WP = SEC + 2, W + 2    # padded section: 58 x 114
NU = SEC // 8              # 8-row elementwise units per half (7)

TAPS = [(ky, kx) for ky in (-1, 0, 1) for kx in (-1, 0, 1)]

LAST_RESULT = None  # BassKernelResults of the most recent kernel() call


def _split_halves(m):
    """[64, 112, X] -> [128, 56, X]: top rows on partitions 0-63, bottom on 64-127."""
    return np.concatenate([m[:, 0:SEC, :], m[:, SEC:H, :]], axis=0)


def _pad_split_image(img):
    """[64,112,112] fp -> [128, 58, 114] f16 padded split layout (1px halo)."""
    p = np.zeros((C, H + 2, W + 2), np.float32)
    p[:, 1:113, 1:113] = img
    top = p[:, 0:HP, :]
    bot = p[:, SEC:SEC + HP, :]
    return np.concatenate([top, bot], axis=0).astype(F16)


def _act_arrays(codes):
    """codes [C*H*W] int32 -> dict of split-layout [128,56,112] f16 arrays."""
    c = codes.reshape(C, H, W)
    s1 = np.select([c == 0, c == 1, c == 2, c == 3], [KREL, 0.0, 2.0, 1.0]).astype(np.float32)
    sc = np.select([c == 0, c == 1], [1.0, 2.0], 0.0).astype(np.float32)
    cd = np.select([c == 2, c == 3], [2.0, 1.0], 0.0).astype(np.float32)
    f = np.where(c == 2, -1.0, 0.0).astype(np.float32)
    return {
        "s1": _split_halves(s1).astype(F16),
        "sc": _split_halves(sc).astype(F16),
        "cd": _split_halves(cd).astype(F16),
        "f": f,  # full [64,112,112] f32 (for host conv fold)
    }


def _conv3x3_host(x, w):
    """x [64,112,112] f32, w [64,64,3,3] f32 -> [64,112,112] f32 (pad 1)."""
    xp = np.zeros((C, H + 2, W + 2), np.float32)
    xp[:, 1:113, 1:113] = x
    out = np.zeros((C, H, W), np.float32)
    for ky in range(3):
        for kx in range(3):
            out += np.tensordot(w[:, :, ky, kx], xp[:, ky:ky + H, kx:kx + W], axes=1)
    return out


# xt DMA row chunks: unit u reads padded rows [8u, 8u+10)
XCHUNKS = [(0, 10)] + [(8 * c + 2, 8 * c + 10) for c in range(1, NU)]


def _build_program():
    nc = bacc.Bacc("TRN2", target_bir_lowering=False, debug=False)

    xin = nc.dram_tensor("xin", [BPC, 128, HP, WP], MDT, kind="ExternalInput")
    w1d = nc.dram_tensor("w1", [9, 128, 128], MDT, kind="ExternalInput")
    w2d = nc.dram_tensor("w2", [9, 128, 128], MDT, kind="ExternalInput")
    mnames = ["s1f", "scf", "cdf", "s1s", "scs", "cds", "g"]
    mdram = {
        k: nc.dram_tensor(k, [128, SEC, W], MDT, kind="ExternalInput") for k in mnames
    }
    outd = nc.dram_tensor("out", [BPC, 128, SEC, W], MDT, kind="ExternalOutput")

    CP = mybir.ActivationFunctionType.Copy
    SG = mybir.ActivationFunctionType.Sigmoid

    with tile.TileContext(nc) as tc, ExitStack() as ctx:
        wp = ctx.enter_context(tc.tile_pool(name="w", bufs=1))
        mp = ctx.enter_context(tc.tile_pool(name="m", bufs=1))
        xp = ctx.enter_context(tc.tile_pool(name="x", bufs=2))
        hp = ctx.enter_context(tc.tile_pool(name="h", bufs=2))
        ep = ctx.enter_context(tc.tile_pool(name="e", bufs=2))
        op_ = ctx.enter_context(tc.tile_pool(name="o", bufs=3))
        pp = ctx.enter_context(tc.tile_pool(name="ps", bufs=4, space="PSUM"))

        w1t = wp.tile([128, 9, 128], MDT, tag="w1")
        w2t = wp.tile([128, 9, 128], MDT, tag="w2")
        for t in range(9):
            nc.sync.dma_start(w1t[:, t, :], w1d[t, :, :])
            nc.sync.dma_start(w2t[:, t, :], w2d[t, :, :])

        mt = {}
        for k in mnames:
            mt[k] = mp.tile([128, SEC, W], MDT, tag=k, name=k)
        # L1 arrays stream in first (unit-interleaved), then the L2 arrays
        for u in range(NU):
            for k in ("s1f", "scf", "cdf"):
                nc.sync.dma_start(mt[k][:, 8 * u:8 * u + 8, :],
                                  mdram[k][:, 8 * u:8 * u + 8, :])
        for u in range(NU):
            for k in ("s1s", "scs", "cds", "g"):
                nc.sync.dma_start(mt[k][:, 8 * u:8 * u + 8, :],
                                  mdram[k][:, 8 * u:8 * u + 8, :])

        def conv_unit(src, wt, ps, r0):
            """9-tap conv into 2-bank psum tile ps[:, 0:8, 0:112] for output
            rows r0..r0+7 of each half; both halves in one matmul via the
            block-diagonal 128x128 weights."""
            for i in (0, 1):
                for t, (ky, kx) in enumerate(TAPS):
                    rs = r0 + 4 * i + 1 + ky
                    nc.tensor.matmul(
                        ps[:, 4 * i:4 * i + 4, 0:112], wt[:, t, :],
                        src[:, rs:rs + 4, kx + 1:kx + 113],
                        start=(t == 0), stop=(t == 8),
                    )

        for n in range(BPC):
            xt = xp.tile([128, HP, WP], MDT, tag="xt")
            for (ra, rb) in XCHUNKS:
                nc.sync.dma_start(xt[:, ra:rb, :], xin[n, :, ra:rb, :])
            ht = hp.tile([128, HP, WP], MDT, tag="ht")
            if n < 2:
                # borders stay zero across reuses; interior is fully rewritten
                nc.gpsimd.memset(ht[:, 0, :], 0.0)
                nc.gpsimd.memset(ht[:, HP - 1, :], 0.0)
                nc.gpsimd.memset(ht[:, :, 0], 0.0)
                nc.gpsimd.memset(ht[:, :, WP - 1], 0.0)

            # ---- layer 1: conv1*a1 -> per-element act -> ht
            for u in range(NU):
                r0 = 8 * u
                ms = slice(r0, r0 + 8)
                ps = pp.tile([128, 8, 128], mybir.dt.float32, tag="ps")
                conv_unit(xt, w1t, ps, r0)
                y1 = ep.tile([128, 8, 112], MDT, tag="y1")
                nc.scalar.activation(y1[:], ps[:, :, 0:112], CP)
                arg = ep.tile([128, 8, 112], MDT, tag="arg")
                nc.vector.tensor_mul(arg[:], y1[:], mt["s1f"][:, ms, :])
                sg = ep.tile([128, 8, 112], MDT, tag="sg")
                nc.scalar.activation(sg[:], arg[:], SG)
                wa = ep.tile([128, 8, 112], MDT, tag="wa")
                nc.gpsimd.tensor_mul(wa[:], y1[:], mt["scf"][:, ms, :])
                wb = ep.tile([128, 8, 112], MDT, tag="wb")
                nc.vector.tensor_add(wb[:], wa[:], mt["cdf"][:, ms, :])
                nc.vector.tensor_mul(ht[:, r0 + 1:r0 + 9, 1:113], sg[:], wb[:])
                if u == 0:
                    # top half's bottom halo row = bottom half's first row
                    nc.gpsimd.dma_start(ht[0:64, HP - 1, 1:113], ht[64:128, 1, 1:113])
            # bottom half's top halo row = top half's last row
            nc.gpsimd.dma_start(ht[64:128, 0, 1:113], ht[0:64, SEC, 1:113])

            # ---- layer 2: conv2*a2 (+ shortcut act(x) + G) -> out
            # u=0 depends on the second halo DMA (after L1 u6): emit it last
            for u in (1, 2, 3, 4, 5, 6, 0):
                r0 = 8 * u
                ms = slice(r0, r0 + 8)
                ps = pp.tile([128, 8, 128], mybir.dt.float32, tag="ps")
                conv_unit(ht, w2t, ps, r0)
                y2 = ep.tile([128, 8, 112], MDT, tag="y2")
                nc.scalar.activation(y2[:], ps[:, :, 0:112], CP)
                xv = xt[:, r0 + 1:r0 + 9, 1:113]
                arg2 = ep.tile([128, 8, 112], MDT, tag="arg2")
                nc.vector.tensor_mul(arg2[:], xv, mt["s1s"][:, ms, :])
                sg2 = ep.tile([128, 8, 112], MDT, tag="sg2")
                nc.scalar.activation(sg2[:], arg2[:], SG)
                wc = ep.tile([128, 8, 112], MDT, tag="wc")
                nc.gpsimd.tensor_mul(wc[:], xv, mt["scs"][:, ms, :])
                wd = ep.tile([128, 8, 112], MDT, tag="wd")
                nc.vector.tensor_add(wd[:], wc[:], mt["cds"][:, ms, :])
                z = ep.tile([128, 8, 112], MDT, tag="z")
                nc.vector.tensor_mul(z[:], sg2[:], wd[:])
                z2 = ep.tile([128, 8, 112], MDT, tag="z2")
                nc.vector.tensor_add(z2[:], z[:], mt["g"][:, ms, :])
                o = op_.tile([128, 8, 112], MDT, tag="o")
                nc.vector.tensor_add(o[:], y2[:], z2[:])
                nc.sync.dma_start(outd[n, :, ms, :], o[:])

    nc.compile()
    return nc


def kernel(x, conv1_w, conv2_w, gamma1, beta1, mean1, var1,
           gamma2, beta2, mean2, var2, act_codes_feat, act_codes_sc):
    x = np.asarray(x, np.float32)
    a1 = (np.asarray(gamma1) / np.sqrt(np.asarray(var1) + EPS)).astype(np.float32)
    b1 = (np.asarray(beta1) - np.asarray(mean1) * a1).astype(np.float32)
    a2 = (np.asarray(gamma2) / np.sqrt(np.asarray(var2) + EPS)).astype(np.float32)
    b2 = (np.asarray(beta2) - np.asarray(mean2) * a2).astype(np.float32)

    # beta1 != 0 would need a per-channel bias on the L1 eviction; the
    # benchmark fills use beta=0, mean=0.
    assert np.allclose(b1, 0.0), "beta1/mean1 fold not implemented for nonzero values"

    mf = _act_arrays(np.asarray(act_codes_feat))
    msk = _act_arrays(np.asarray(act_codes_sc))

    w1s = np.asarray(conv1_w, np.float32) * a1[:, None, None, None]
    w2s = np.asarray(conv2_w, np.float32) * a2[:, None, None, None]

    # L1's "+F" is linear through conv2: fold conv2(ff) plus the L2-side
    # constants (f2, beta2) into one G array added on the output.
    k2 = _conv3x3_host(mf["f"], w2s)
    g = k2 + msk["f"] + b2[:, None, None]

    w1h = np.zeros((9, 128, 128), F16)
    w2h = np.zeros((9, 128, 128), F16)
    for t, (ky, kx) in enumerate(TAPS):
        wt1 = w1s[:, :, ky + 1, kx + 1].T.astype(F16)
        wt2 = w2s[:, :, ky + 1, kx + 1].T.astype(F16)
        w1h[t, 0:64, 0:64] = wt1
        w1h[t, 64:128, 64:128] = wt1
        w2h[t, 0:64, 0:64] = wt2
        w2h[t, 64:128, 64:128] = wt2

    nc = _build_program()

    in_maps = []
    for core in range(NCORES):
        xs = np.stack([
            _pad_split_image(x[core * BPC + i]) for i in range(BPC)
        ])
        in_maps.append({
            "xin": xs,
            "w1": w1h, "w2": w2h,
            "s1f": mf["s1"], "scf": mf["sc"], "cdf": mf["cd"],
            "s1s": msk["s1"], "scs": msk["sc"], "cds": msk["cd"],
            "g": _split_halves(g).astype(F16),
        })

    res = run_bass_kernel_spmd(nc, in_maps, core_ids=list(range(NCORES)))
    global LAST_RESULT
    LAST_RESULT = res

    out = np.empty((B, C, H, W), np.float32)
    for core in range(NCORES):
        o = res.results[core]["out"]  # [BPC, 128, 56, 112] f16
        for i in range(BPC):
            img = np.concatenate([o[i, 0:64], o[i, 64:128]], axis=1)
            out[core * BPC + i] = img.astype(np.float32)
    return out


if __name__ == "__main__":
    rng = np.random.default_rng(0)
    inputs = {
        "x": rng.standard_normal((B, C, H, W), np.float32),
        "conv1_w": rng.standard_normal((C, C, 3, 3), np.float32) * 0.05,
        "conv2_w": rng.standard_normal((C, C, 3, 3), np.float32) * 0.05,
        "gamma1": np.ones(C, np.float32), "beta1": np.zeros(C, np.float32),
        "mean1": np.zeros(C, np.float32), "var1": np.ones(C, np.float32),
        "gamma2": np.ones(C, np.float32), "beta2": np.zeros(C, np.float32),
        "mean2": np.zeros(C, np.float32), "var2": np.ones(C, np.float32),
        "act_codes_feat": rng.integers(0, 4, C * H * W).astype(np.int32),
        "act_codes_sc": rng.integers(0, 4, C * H * W).astype(np.int32),
    }
    out = kernel(**inputs)
    print("out", out.shape, out.dtype, float(np.abs(out).max()))


# revision 10
# speedup vs baseline: 2.2978x; 1.2369x over previous
"""Trainium2 Bass kernel for nn_BasicBlock (conv3x3-BN-perelem_act-conv3x3-BN + act shortcut).

Data-parallel over batch: 32 images -> 4 per core x 8 cores.

Per-core layout: each 64x112x112 image is split into top/bottom 56-row halves,
mapped to SBUF partitions 0-63 (top, one per channel) and 64-127 (bottom), so
every elementwise op runs with all 128 lanes and the per-element activation
mask arrays need only a single copy.

Conv3x3 = 9 accumulating matmuls per 8-row output chunk, each using the FULL
128x128 PE array via block-diagonal weights: W128[t] = diag(W_t, W_t) so one
instruction computes tap t for both halves (4.5 matmul-rows per output element
-- the K=128 packing floor for a 64-channel 3x3 conv).

BN is folded entirely into the weights (scale) and constant fields (shift).

Per-element activation (codes 0..3 = relu/identity/tanh/sigmoid) is computed
without any predicated copy:
    act(y) = sigmoid(s1*y) * (y*SC + CD) + F
  s1 = {relu: 512, id: 0, tanh: 2, sigmoid: 1}   (sigmoid(0)=0.5 covers id)
  SC = {relu: 1, id: 2, tanh: 0, sigmoid: 0}
  CD = {tanh: 2, sigmoid: 1, else 0}
  F  = {tanh: -1, else 0}
The L1 "+F" is linear through conv2, so it is folded host-side into
K2 = conv2(ff) and merged with the L2 constants into a single G array:
    out = y2 + sigmoid(s1s*x)*(x*SCs + CDs) + G,   G = K2 + f2 + beta2-fold
The shortcut reads x from the SBUF-resident input tile (no reload DMA).
"""

import os
import sys

sys.path.insert(0, "/opt/trn_rl_repo")

import numpy as np
from contextlib import ExitStack

import concourse.bass as bass
import concourse.bacc as bacc
import concourse.tile as tile
import concourse.mybir as mybir
from concourse.bass_utils import run_bass_kernel_spmd

F16 = np.float16
MDT = mybir.dt.float16
EPS = 1e-5
KREL = 512.0   # sigmoid(KREL*y) ~ step(y) for the relu branch

B, C, H, W = 32, 64, 112, 112
NCORES = 8
BPC = B // NCORES          # images per core
SEC = H // 2               # rows per half-section (56)
HP, WP = SEC + 2, W + 2    # padded section: 58 x 114
NU = SEC // 8              # 8-row elementwise units per half (7)

TAPS = [(ky, kx) for ky in (-1, 0, 1) for kx in (-1, 0, 1)]

LAST_RESULT = None  # BassKernelResults of the most recent kernel() call


def _split_halves(m):
    """[64, 112, X] -> [128, 56, X]: top rows on partitions 0-63, bottom on 64-127."""
    return np.concatenate([m[:, 0:SEC, :], m[:, SEC:H, :]], axis=0)


def _pad_split_image(img):
    """[64,112,112] fp -> [128, 58, 114] f16 padded split layout (1px halo)."""
    p = np.zeros((C, H + 2, W + 2), np.float32)
    p[:, 1:113, 1:113] = img
    top = p[:, 0:HP, :]
    bot = p[:, SEC:SEC + HP, :]
    return np.concatenate([top, bot], axis=0).astype(F16)


def _act_arrays(codes):
    """codes [C*H*W] int32 -> dict of split-layout [128,56,112] f16 arrays."""
    c = codes.reshape(C, H, W)
    s1 = np.select([c == 0, c == 1, c == 2, c == 3], [KREL, 0.0, 2.0, 1.0]).astype(np.float32)
    sc = np.select([c == 0, c == 1], [1.0, 2.0], 0.0).astype(np.float32)
    cd = np.select([c == 2, c == 3], [2.0, 1.0], 0.0).astype(np.float32)
    f = np.where(c == 2, -1.0, 0.0).astype(np.float32)
    return {
        "s1": _split_halves(s1).astype(F16),
        "sc": _split_halves(sc).astype(F16),
        "cd": _split_halves(cd).astype(F16),
        "f": f,  # full [64,112,112] f32 (for host conv fold)
    }


def _conv3x3_host(x, w):
    """x [64,112,112] f32, w [64,64,3,3] f32 -> [64,112,112] f32 (pad 1)."""
    xp = np.zeros((C, H + 2, W + 2), np.float32)
    xp[:, 1:113, 1:113] = x
    out = np.zeros((C, H, W), np.float32)
    for ky in range(3):
        for kx in range(3):
            out += np.tensordot(w[:, :, ky, kx], xp[:, ky:ky + H, kx:kx + W], axes=1)
    return out


# xt DMA row chunks: unit u reads padded rows [8u, 8u+10)
XCHUNKS = [(0, 10)] + [(8 * c + 2, 8 * c + 10) for c in range(1, NU)]


def _build_program():
    nc = bacc.Bacc("TRN2", target_bir_lowering=False, debug=False)

    xin = nc.dram_tensor("xin", [BPC, 128, HP, WP], MDT, kind="ExternalInput")
    w1d = nc.dram_tensor("w1", [128, 9, 128], MDT, kind="ExternalInput")
    w2d = nc.dram_tensor("w2", [128, 9, 128], MDT, kind="ExternalInput")
    mnames = ["s1f", "scf", "cdf", "s1s", "scs", "cds", "g"]
    mdram = {
        k: nc.dram_tensor(k, [128, SEC, W], MDT, kind="ExternalInput") for k in mnames
    }
    outd = nc.dram_tensor("out", [BPC, 128, SEC, W], MDT, kind="ExternalOutput")

    CP = mybir.ActivationFunctionType.Copy
    SG = mybir.ActivationFunctionType.Sigmoid

    with tile.TileContext(nc) as tc, ExitStack() as ctx:
        wp = ctx.enter_context(tc.tile_pool(name="w", bufs=1))
        mp = ctx.enter_context(tc.tile_pool(name="m", bufs=1))
        xp = ctx.enter_context(tc.tile_pool(name="x", bufs=2))
        hp = ctx.enter_context(tc.tile_pool(name="h", bufs=2))
        ep = ctx.enter_context(tc.tile_pool(name="e", bufs=2))
        op_ = ctx.enter_context(tc.tile_pool(name="o", bufs=3))
        pp = ctx.enter_context(tc.tile_pool(name="ps", bufs=4, space="PSUM"))

        w1t = wp.tile([128, 9, 128], MDT, tag="w1")
        w2t = wp.tile([128, 9, 128], MDT, tag="w2")
        mt = {}
        for k in mnames:
            mt[k] = mp.tile([128, SEC, W], MDT, tag=k, name=k)

        def mchunk(k, u):
            nc.sync.dma_start(mt[k][:, 8 * u:8 * u + 8, :],
                              mdram[k][:, 8 * u:8 * u + 8, :])

        # Startup DMA order is the SP-queue order: image-0's first input chunk
        # and unit-0 L1 masks must land first so PE starts within ~4us.
        xt0 = xp.tile([128, HP, WP], MDT, tag="xt")
        nc.sync.dma_start(w1t[:], w1d[:])
        nc.sync.dma_start(xt0[:, 0:10, :], xin[0, :, 0:10, :])
        for k in ("s1f", "scf", "cdf"):
            mchunk(k, 0)
        nc.sync.dma_start(w2t[:], w2d[:])
        for u in range(1, NU):
            ra, rb = XCHUNKS[u]
            nc.sync.dma_start(xt0[:, ra:rb, :], xin[0, :, ra:rb, :])
            for k in ("s1f", "scf", "cdf"):
                mchunk(k, u)
        for u in range(NU):
            for k in ("s1s", "scs", "cds", "g"):
                mchunk(k, u)

        def conv_unit(src, wt, ps, r0):
            """9-tap conv into 2-bank psum tile ps[:, 0:8, 0:112] for output
            rows r0..r0+7 of each half; both halves in one matmul via the
            block-diagonal 128x128 weights."""
            for i in (0, 1):
                for t, (ky, kx) in enumerate(TAPS):
                    rs = r0 + 4 * i + 1 + ky
                    nc.tensor.matmul(
                        ps[:, 4 * i:4 * i + 4, 0:112], wt[:, t, :],
                        src[:, rs:rs + 4, kx + 1:kx + 113],
                        start=(t == 0), stop=(t == 8),
                    )

        for n in range(BPC):
            if n == 0:
                xt = xt0
            else:
                xt = xp.tile([128, HP, WP], MDT, tag="xt")
                for (ra, rb) in XCHUNKS:
                    nc.sync.dma_start(xt[:, ra:rb, :], xin[n, :, ra:rb, :])
            ht = hp.tile([128, HP, WP], MDT, tag="ht")
            if n < 2:
                # borders stay zero across reuses; interior is fully rewritten
                nc.gpsimd.memset(ht[:, 0, :], 0.0)
                nc.gpsimd.memset(ht[:, HP - 1, :], 0.0)
                nc.gpsimd.memset(ht[:, :, 0], 0.0)
                nc.gpsimd.memset(ht[:, :, WP - 1], 0.0)

            # ---- layer 1: conv1*a1 -> per-element act -> ht
            for u in range(NU):
                r0 = 8 * u
                ms = slice(r0, r0 + 8)
                ps = pp.tile([128, 8, 128], mybir.dt.float32, tag="ps")
                conv_unit(xt, w1t, ps, r0)
                y1 = ep.tile([128, 8, 112], MDT, tag="y1")
                nc.scalar.activation(y1[:], ps[:, :, 0:112], CP)
                arg = ep.tile([128, 8, 112], MDT, tag="arg")
                nc.vector.tensor_mul(arg[:], y1[:], mt["s1f"][:, ms, :])
                sg = ep.tile([128, 8, 112], MDT, tag="sg")
                nc.scalar.activation(sg[:], arg[:], SG)
                wa = ep.tile([128, 8, 112], MDT, tag="wa")
                nc.gpsimd.tensor_mul(wa[:], y1[:], mt["scf"][:, ms, :])
                wb = ep.tile([128, 8, 112], MDT, tag="wb")
                nc.vector.tensor_add(wb[:], wa[:], mt["cdf"][:, ms, :])
                nc.vector.tensor_mul(ht[:, r0 + 1:r0 + 9, 1:113], sg[:], wb[:])
                if u == 0:
                    # top half's bottom halo row = bottom half's first row
                    nc.gpsimd.dma_start(ht[0:64, HP - 1, 1:113], ht[64:128, 1, 1:113])
            # bottom half's top halo row = top half's last row
            nc.gpsimd.dma_start(ht[64:128, 0, 1:113], ht[0:64, SEC, 1:113])

            # ---- layer 2: conv2*a2 (+ shortcut act(x) + G) -> out
            # u=0 depends on the second halo DMA (after L1 u6): emit it last
            for u in (1, 2, 3, 4, 5, 6, 0):
                r0 = 8 * u
                ms = slice(r0, r0 + 8)
                # shortcut chain only needs xt: emit before the conv so the
                # post-matmul tail is just evict -> add -> DMA
                xv = xt[:, r0 + 1:r0 + 9, 1:113]
                arg2 = ep.tile([128, 8, 112], MDT, tag="arg2")
                nc.vector.tensor_mul(arg2[:], xv, mt["s1s"][:, ms, :])
                sg2 = ep.tile([128, 8, 112], MDT, tag="sg2")
                nc.scalar.activation(sg2[:], arg2[:], SG)
                wc = ep.tile([128, 8, 112], MDT, tag="wc")
                nc.gpsimd.tensor_mul(wc[:], xv, mt["scs"][:, ms, :])
                wd = ep.tile([128, 8, 112], MDT, tag="wd")
                nc.vector.tensor_add(wd[:], wc[:], mt["cds"][:, ms, :])
                z = ep.tile([128, 8, 112], MDT, tag="z")
                nc.vector.tensor_mul(z[:], sg2[:], wd[:])
                z2 = ep.tile([128, 8, 112], MDT, tag="z2")
                nc.vector.tensor_add(z2[:], z[:], mt["g"][:, ms, :])
                ps = pp.tile([128, 8, 128], mybir.dt.float32, tag="ps")
                conv_unit(ht, w2t, ps, r0)
                y2 = ep.tile([128, 8, 112], MDT, tag="y2")
                nc.scalar.activation(y2[:], ps[:, :, 0:112], CP)
                o = op_.tile([128, 8, 112], MDT, tag="o")
                nc.vector.tensor_add(o[:], y2[:], z2[:])
                nc.sync.dma_start(outd[n, :, ms, :], o[:])

    nc.compile()
    return nc


def kernel(x, conv1_w, conv2_w, gamma1, beta1, mean1, var1,
           gamma2, beta2, mean2, var2, act_codes_feat, act_codes_sc):
    x = np.asarray(x, np.float32)
    a1 = (np.asarray(gamma1) / np.sqrt(np.asarray(var1) + EPS)).astype(np.float32)
    b1 = (np.asarray(beta1) - np.asarray(mean1) * a1).astype(np.float32)
    a2 = (np.asarray(gamma2) / np.sqrt(np.asarray(var2) + EPS)).astype(np.float32)
    b2 = (np.asarray(beta2) - np.asarray(mean2) * a2).astype(np.float32)

    # beta1 != 0 would need a per-channel bias on the L1 eviction; the
    # benchmark fills use beta=0, mean=0.
    assert np.allclose(b1, 0.0), "beta1/mean1 fold not implemented for nonzero values"

    mf = _act_arrays(np.asarray(act_codes_feat))
    msk = _act_arrays(np.asarray(act_codes_sc))

    w1s = np.asarray(conv1_w, np.float32) * a1[:, None, None, None]
    w2s = np.asarray(conv2_w, np.float32) * a2[:, None, None, None]

    # L1's "+F" is linear through conv2: fold conv2(ff) plus the L2-side
    # constants (f2, beta2) into one G array added on the output.
    k2 = _conv3x3_host(mf["f"], w2s)
    g = k2 + msk["f"] + b2[:, None, None]

    w1h = np.zeros((128, 9, 128), F16)
    w2h = np.zeros((128, 9, 128), F16)
    for t, (ky, kx) in enumerate(TAPS):
        wt1 = w1s[:, :, ky + 1, kx + 1].T.astype(F16)
        wt2 = w2s[:, :, ky + 1, kx + 1].T.astype(F16)
        w1h[0:64, t, 0:64] = wt1
        w1h[64:128, t, 64:128] = wt1
        w2h[0:64, t, 0:64] = wt2
        w2h[64:128, t, 64:128] = wt2

    nc = _build_program()

    in_maps = []
    for core in range(NCORES):
        xs = np.stack([
            _pad_split_image(x[core * BPC + i]) for i in range(BPC)
        ])
        in_maps.append({
            "xin": xs,
            "w1": w1h, "w2": w2h,
            "s1f": mf["s1"], "scf": mf["sc"], "cdf": mf["cd"],
            "s1s": msk["s1"], "scs": msk["sc"], "cds": msk["cd"],
            "g": _split_halves(g).astype(F16),
        })

    res = run_bass_kernel_spmd(nc, in_maps, core_ids=list(range(NCORES)))
    global LAST_RESULT
    LAST_RESULT = res

    out = np.empty((B, C, H, W), np.float32)
    for core in range(NCORES):
        o = res.results[core]["out"]  # [BPC, 128, 56, 112] f16
        for i in range(BPC):
            img = np.concatenate([o[i, 0:64], o[i, 64:128]], axis=1)
            out[core * BPC + i] = img.astype(np.float32)
    return out


if __name__ == "__main__":
    rng = np.random.default_rng(0)
    inputs = {
        "x": rng.standard_normal((B, C, H, W), np.float32),
        "conv1_w": rng.standard_normal((C, C, 3, 3), np.float32) * 0.05,
        "conv2_w": rng.standard_normal((C, C, 3, 3), np.float32) * 0.05,
        "gamma1": np.ones(C, np.float32), "beta1": np.zeros(C, np.float32),
        "mean1": np.zeros(C, np.float32), "var1": np.ones(C, np.float32),
        "gamma2": np.ones(C, np.float32), "beta2": np.zeros(C, np.float32),
        "mean2": np.zeros(C, np.float32), "var2": np.ones(C, np.float32),
        "act_codes_feat": rng.integers(0, 4, C * H * W).astype(np.int32),
        "act_codes_sc": rng.integers(0, 4, C * H * W).astype(np.int32),
    }
    out = kernel(**inputs)
    print("out", out.shape, out.dtype, float(np.abs(out).max()))
